# revision 1
# baseline (speedup 1.0000x reference)
"""Diagonally-masked multi-head self-attention on 8 Trainium2 NeuronCores.

Problem (full shapes): x [2,2048,512], wq/wk/wv [512,512], wo [512,512],
H=8 heads, Dh=64.  out = softmax(mask_diag(q k^T / 8)) v @ wo.

Sharding: core c handles batch b = c//4 and head pair g = c%4
(heads 2g, 2g+1).  Each core computes its two heads' attention and a
partial output  y_c = sum_h (O_h / d_h) @ wo[h rows]  for its batch;
the host sums the 4 partials per batch (row-sharded wo all-reduce done
at gather time).

Per-core kernel dataflow (all matmuls bf16 with fp32 PSUM accumulation):
  xt  = x[b].T                          (uploaded pre-transposed, bf16)
  QK_h = [wq_h/8 | wk_h].T @ xt         -> [128, L]  (rows 0:64 Q^T, 64:128 K^T)
  V    = xt.T @ [wv_h0|wv_h1]           -> per key tile [128, 130] with a
                                           ones column appended per head
  S^T  = K Q^T (per 128-key tile)       -> PSUM, exp on ACT -> P^T (bf16)
  diag: P^T diagonal block zeroed via (1-I) mask multiply
  O'^T = V'^T P^T accumulated over key tiles -> [65, L] PSUM
         (row 64 = softmax denominator d, since V' col 64/129 is ones)
  y_h  = (O_h @ wo_h) * (1/d)  summed over the 2 heads on DVE.

The softmax is computed without max-subtraction: scores are ~N(0, 0.04)
(|s| < ~1.3), so exp never overflows; the diagonal -inf mask becomes a
multiply-by-zero after exp.
"""

import sys

if "/opt/trn_rl_repo" not in sys.path:
    sys.path.insert(0, "/opt/trn_rl_repo")

import numpy as np
import ml_dtypes

import concourse.bacc as bacc
import concourse.tile as tile
from concourse import mybir
from concourse.bass_utils import run_bass_kernel_spmd

N_CORES = 8
B, L, D = 2, 2048, 512
H, DH = 8, 64
HEADS_PER_CORE = 2
NKT = L // 128  # 16 key/query tiles
BF16 = mybir.dt.bfloat16
F32 = mybir.dt.float32
F32R = mybir.dt.float32r

# test.py can flip these before calling kernel()
TRACE = False
_LAST_RESULTS = {}

_NC_CACHE = {}


def _build_nc():
    nc = bacc.Bacc(
        "TRN2",
        target_bir_lowering=False,
        debug=False,
        enable_asserts=False,
        num_devices=N_CORES,
    )
    xt = nc.dram_tensor("xt", [D, L], BF16, kind="ExternalInput").ap()
    wqk = nc.dram_tensor("wqk", [D, 256], BF16, kind="ExternalInput").ap()
    wv = nc.dram_tensor("wv", [D, 128], BF16, kind="ExternalInput").ap()
    wo = nc.dram_tensor("wo", [128, D], BF16, kind="ExternalInput").ap()
    msk = nc.dram_tensor("msk", [128, 128], BF16, kind="ExternalInput").ap()
    y = nc.dram_tensor("y", [L, D], F32, kind="ExternalOutput").ap()
    dscr = nc.dram_tensor("dscr", [4, L // 2], F32, kind="Internal").ap()

    with tile.TileContext(nc) as tc:
        _emit(nc, tc, xt, wqk, wv, wo, msk, y, dscr)
    nc.compile()
    return nc


def _emit(nc, tc, xt, wqk, wv, wo, msk, y, dscr):
    import contextlib

    HQ = L // 2  # 1024 queries per half

    ctx = contextlib.ExitStack()
    with ctx:
        singles = ctx.enter_context(tc.tile_pool(name="singles", bufs=1))
        ptp = ctx.enter_context(tc.tile_pool(name="pt", bufs=6))
        ysb = ctx.enter_context(tc.tile_pool(name="ysb", bufs=6))
        dbcp = ctx.enter_context(tc.tile_pool(name="dbcp", bufs=2))
        otmpp = ctx.enter_context(tc.tile_pool(name="otmpp", bufs=3))
        # PSUM budget (8 banks): psmm 2x[128,1024]=4, psacc 1x[128,1024]=2,
        # psaux 2x[128,512]=2.  psmm: S tiles + initial QK-h0 proj only.
        # psacc: the O'^T accumulator.  psaux: V proj, QK-h1 proj, wo.
        psmm = ctx.enter_context(tc.tile_pool(name="psmm", bufs=2, space="PSUM"))
        psacc = ctx.enter_context(tc.tile_pool(name="psacc", bufs=1, space="PSUM"))
        psaux = ctx.enter_context(tc.tile_pool(name="psaux", bufs=2, space="PSUM"))

        # warm the ACT exp table set before anything depends on ACT (a real
        # table load is ~2.7us; Copy works from any set, exp does not)
        warm = singles.tile([1, 4], F32, tag="warm", name="warm")
        nc.vector.memset(warm, 0.0)
        nc.scalar.activation(warm, warm, mybir.ActivationFunctionType.Exp)

        # ---- loads (consumption order; xt split for earlier first-use) ----
        wqk_sb = []
        for c in range(4):
            t = singles.tile([128, 256], BF16, tag=f"wqk{c}", name=f"wqk{c}")
            nc.sync.dma_start(out=t, in_=wqk[c * 128 : (c + 1) * 128, :])
            wqk_sb.append(t)
        xt_sb = [
            singles.tile([128, L], BF16, tag=f"xt{c}", name=f"xt{c}") for c in range(4)
        ]
        for c in range(4):
            nc.sync.dma_start(
                out=xt_sb[c][:, 0:512], in_=xt[c * 128 : (c + 1) * 128, 0:512]
            )
        wv_sb = []
        for c in range(4):
            t = singles.tile([128, 128], BF16, tag=f"wv{c}", name=f"wv{c}")
            nc.sync.dma_start(out=t, in_=wv[c * 128 : (c + 1) * 128, :])
            wv_sb.append(t)
        msk_sb = singles.tile([128, 128], BF16, tag="msk", name="msk_sb")
        nc.sync.dma_start(out=msk_sb, in_=msk)
        for q4 in range(1, 4):
            for c in range(4):
                nc.sync.dma_start(
                    out=xt_sb[c][:, q4 * 512 : (q4 + 1) * 512],
                    in_=xt[c * 128 : (c + 1) * 128, q4 * 512 : (q4 + 1) * 512],
                )
        wo_sb = singles.tile([128, D], BF16, tag="wo", name="wo_sb")
        nc.sync.dma_start(out=wo_sb, in_=wo)

        q_sb = [singles.tile([64, L], BF16, tag=f"q{h}", name=f"q{h}") for h in range(2)]
        k_sb = [singles.tile([64, L], BF16, tag=f"k{h}", name=f"k{h}") for h in range(2)]

        def qk_proj(h, pool, ptag, k_on_act):
            # PSUM rows 0:64 are Q^T, 64:128 K^T; partition-shifted copy for K^T
            for nt in range(4):
                ps = pool.tile([128, 512], F32, tag=ptag, name="qkps", padded_shape=[128, 1024] if ptag == "mm" else None)
                for kc in range(4):
                    nc.tensor.matmul(
                        ps,
                        lhsT=wqk_sb[kc][:, h * 128 : (h + 1) * 128],
                        rhs=xt_sb[kc][:, nt * 512 : (nt + 1) * 512],
                        start=(kc == 0),
                        stop=(kc == 3),
                    )
                nc.vector.tensor_copy(q_sb[h][:, nt * 512 : (nt + 1) * 512], ps[0:64, :])
                kdst = k_sb[h][:, nt * 512 : (nt + 1) * 512]
                if k_on_act:
                    nc.scalar.copy(kdst, ps[64:128, :])
                else:
                    nc.vector.tensor_copy(kdst, ps[64:128, :])

        qk_proj(0, psmm, "mm", True)

        # ---- V projection (emission interleaved into the first kt loop) ----
        v_sb = [singles.tile([128, 130], BF16, tag=f"v{lt}", name=f"v{lt}") for lt in range(NKT)]

        def v_proj(lt):
            ps = psaux.tile([128, 128], F32, tag="aux", name="vps", padded_shape=[128, 512])
            for kc in range(4):
                nc.tensor.matmul(
                    ps,
                    lhsT=xt_sb[kc][:, lt * 128 : (lt + 1) * 128],
                    rhs=wv_sb[kc],
                    start=(kc == 0),
                    stop=(kc == 3),
                )
            nc.vector.tensor_copy(v_sb[lt][:, 0:64], ps[:, 0:64])
            nc.vector.tensor_copy(v_sb[lt][:, 65:129], ps[:, 64:128])
            nc.vector.memset(v_sb[lt][:, 64:65], 1.0)
            nc.vector.memset(v_sb[lt][:, 129:130], 1.0)

        for lt in range(6):
            v_proj(lt)

        # ---- attention; O^T normalized by 1/d after a fast PSUM drain ----
        # OT_all rows 0:64 = head0 O^T/d, rows 64:128 = head1 O^T/d
        ot_all = singles.tile([128, L], BF16, tag="ot", name="ot_all")
        drow_sb = [
            singles.tile([1, HQ], F32, tag=f"dr{i}", name=f"dr{i}") for i in range(4)
        ]
        for h in range(2):
            for hf in range(2):
                po = psacc.tile([65, HQ], F32, tag="acc", name="acc")
                for kt in range(NKT):
                    pt = ptp.tile([128, HQ], BF16, tag="pt", name="pt")
                    ps = psmm.tile([128, HQ], F32, tag="mm", name="mm")
                    for nt in range(2):
                        nc.tensor.matmul(
                            ps[:, nt * 512 : (nt + 1) * 512],
                            lhsT=k_sb[h][:, kt * 128 : (kt + 1) * 128],
                            rhs=q_sb[h][
                                :, hf * HQ + nt * 512 : hf * HQ + (nt + 1) * 512
                            ],
                            start=True,
                            stop=True,
                        )
                    nc.scalar.activation(pt, ps, mybir.ActivationFunctionType.Exp)
                    if kt // 8 == hf:
                        off = (kt % 8) * 128
                        nc.vector.tensor_mul(
                            pt[:, off : off + 128], pt[:, off : off + 128], msk_sb
                        )
                    for nt in range(2):
                        nc.tensor.matmul(
                            po[:, nt * 512 : (nt + 1) * 512],
                            lhsT=v_sb[kt][:, h * 65 : (h + 1) * 65],
                            rhs=pt[:, nt * 512 : (nt + 1) * 512],
                            start=(kt == 0),
                            stop=(kt == NKT - 1),
                        )
                    if h == 0 and hf == 0 and 0 < kt + 8 - 1 and kt + 8 < NKT + 1 and kt < 9 and kt + 7 < NKT:
                        v_proj(kt + 7)
                        if kt == 0:
                            v_proj(6)
                # fast drain so the accumulator frees quickly; normalize later
                i = 2 * h + hf
                otmp = otmpp.tile([64, HQ], F32, tag="otmp", name="otmp")
                nc.scalar.copy(otmp, po[0:64, :])
                nc.vector.reciprocal(drow_sb[i], po[64:65, :])
                nc.sync.dma_start(out=dscr[i : i + 1, :], in_=drow_sb[i])
                rbc = dbcp.tile([64, HQ], F32, tag="rbc", name="rbc")
                nc.sync.dma_start(
                    out=rbc, in_=dscr[i : i + 1, :].to_broadcast([64, HQ])
                )
                nc.vector.tensor_mul(
                    ot_all[h * 64 : (h + 1) * 64, hf * HQ : (hf + 1) * HQ],
                    otmp,
                    rbc,
                )
                if h == 0 and hf == 0:
                    qk_proj(1, psaux, "aux", False)

        # ---- output projection: y = (O/d | both heads) @ wo ----
        for lt in range(NKT):
            pool, ptag = (psaux, "aux") if lt % 2 == 0 else (psacc, "acc")
            psy = pool.tile(
                [128, 512], F32, tag=ptag, name="psy",
                padded_shape=[128, HQ] if ptag == "acc" else None,
            )
            nc.tensor.matmul(
                psy,
                lhsT=ot_all[:, lt * 128 : (lt + 1) * 128],
                rhs=wo_sb,
                start=True,
                stop=True,
            )
            yt = ysb.tile([128, 512], F32, tag="yt", name="yt")
            if lt % 2 == 0:
                nc.vector.tensor_copy(yt, psy)
            else:
                nc.scalar.copy(yt, psy)
            nc.sync.dma_start(out=y[lt * 128 : (lt + 1) * 128, :], in_=yt)


def _get_nc():
    if "nc" not in _NC_CACHE:
        _NC_CACHE["nc"] = _build_nc()
    return _NC_CACHE["nc"]


def kernel(x, wq, wk, wv, wo):
    x = np.asarray(x, dtype=np.float32)
    wq = np.asarray(wq, dtype=np.float32)
    wk = np.asarray(wk, dtype=np.float32)
    wv = np.asarray(wv, dtype=np.float32)
    wo = np.asarray(wo, dtype=np.float32)

    scale = 1.0 / (DH**0.5)
    bf = ml_dtypes.bfloat16
    msk = (1.0 - np.eye(128, dtype=np.float32)).astype(bf)

    in_maps = []
    for c in range(N_CORES):
        b, g = divmod(c, 4)
        h0, h1 = 2 * g, 2 * g + 1
        wqk_c = np.concatenate(
            [
                wq[:, h0 * DH : (h0 + 1) * DH] * scale,
                wk[:, h0 * DH : (h0 + 1) * DH],
                wq[:, h1 * DH : (h1 + 1) * DH] * scale,
                wk[:, h1 * DH : (h1 + 1) * DH],
            ],
            axis=1,
        )
        wv_c = wv[:, h0 * DH : (h1 + 1) * DH]
        wo_c = wo[h0 * DH : (h1 + 1) * DH, :]
        in_maps.append(
            {
                "xt": np.ascontiguousarray(x[b].T).astype(bf),
                "wqk": wqk_c.astype(bf),
                "wv": np.ascontiguousarray(wv_c).astype(bf),
                "wo": np.ascontiguousarray(wo_c).astype(bf),
                "msk": msk,
            }
        )

    nc = _get_nc()
    res = run_bass_kernel_spmd(
        nc, in_maps, core_ids=list(range(N_CORES)), trace=TRACE
    )
    _LAST_RESULTS["res"] = res

    out = np.empty((B, L, D), dtype=np.float32)
    for b in range(B):
        acc = res.results[4 * b]["y"].astype(np.float32).copy()
        for g in range(1, 4):
            acc += res.results[4 * b + g]["y"]
        out[b] = acc
    return out



# revision 4
# speedup vs baseline: 4.7748x; 4.7748x over previous
"""Diagonally-masked multi-head self-attention on 8 Trainium2 NeuronCores.

Problem (full shapes): x [2,2048,512], wq/wk/wv [512,512], wo [512,512],
H=8 heads, Dh=64.  out = softmax(mask_diag(q k^T / 8)) v @ wo.

The axon tunnel (~30-40MB/s host<->device) dominates wall time, so the
split minimizes bytes moved:
  host:   q/k/v projections (fp32 BLAS GEMMs, ~65ms) -> per-core bf16
          packs.  Final  y = (O/d) @ wo  GEMM also on host (~21ms).
  device: only the O(L^2) attention.  Core c handles batch b=c//4 and
          head pair g=c%4 (heads 2g, 2g+1):
            S^T = K Q^T (scale folded into q on host), exp on ACT,
            diagonal block zeroed via (1-I) mask multiply,
            O'^T = V'^T P^T accumulated over 16 key tiles in PSUM,
            where V' has a ones column per head so row 64 of the
            accumulator is the softmax denominator d.
          Outputs per core are disjoint (no partial-sum all-reduce):
          ot [128,2048] bf16 (unnormalized O'^T, both heads) and
          dd [2,2048] f32 (d per head).
  upload ~12.3MB/call (qk 8MB + v_aug 4.3MB), download ~4.1MB.

Dispatch bypasses run_bass_kernel_spmd: the jitted shard_map'd bass_exec
call is built once and cached; the (1-I) mask constant and the output
placeholder operands live on device permanently, so per-call transfers
are inputs+outputs only.  exp needs no max-subtraction: scores are
~N(0, 0.04) so |s| < ~1.3.
"""

import sys

if "/opt/trn_rl_repo" not in sys.path:
    sys.path.insert(0, "/opt/trn_rl_repo")

import numpy as np
import ml_dtypes

import jax
from jax.experimental.shard_map import shard_map
from jax.sharding import Mesh, NamedSharding, PartitionSpec as P

import concourse.bacc as bacc
import concourse.tile as tile
from concourse import mybir
from concourse import bass2jax as _b2j

N_CORES = 8
B, L, D = 2, 2048, 512
H, DH = 8, 64
HQ = L // 2  # 1024 queries per half
NKT = L // 128  # 16 key tiles
BF16 = mybir.dt.bfloat16
F32 = mybir.dt.float32
BF = ml_dtypes.bfloat16

# test.py compatibility
TRACE = False
_LAST_RESULTS = {}

_CTX = {}


def _build_nc():
    nc = bacc.Bacc(
        "TRN2",
        target_bir_lowering=False,
        debug=False,
        enable_asserts=False,
        num_devices=N_CORES,
    )
    # per-core: qk rows 0:64 q_h0^T (scale folded), 64:128 k_h0^T,
    # 128:192 q_h1^T, 192:256 k_h1^T
    qk = nc.dram_tensor("qk", [256, L], BF16, kind="ExternalInput").ap()
    va = nc.dram_tensor("va", [L, 130], BF16, kind="ExternalInput").ap()
    msk = nc.dram_tensor("msk", [128, 128], BF16, kind="ExternalInput").ap()
    ot = nc.dram_tensor("ot", [128, L], BF16, kind="ExternalOutput").ap()
    dd = nc.dram_tensor("dd", [2, L], F32, kind="ExternalOutput").ap()
    with tile.TileContext(nc) as tc:
        _emit(nc, tc, qk, va, msk, ot, dd)
    nc.compile()
    return nc


def _emit(nc, tc, qk, va, msk, ot, dd):
    import contextlib

    ctx = contextlib.ExitStack()
    with ctx:
        singles = ctx.enter_context(tc.tile_pool(name="singles", bufs=1))
        ptp = ctx.enter_context(tc.tile_pool(name="pt", bufs=4))
        # PSUM budget (8 banks): psmm 2x[128,1024]=4 (S^T tiles), psacc
        # 2x[65,1024]=4 (the O'^T accumulator, double-buffered across
        # the 4 (h,hf) iterations).
        psmm = ctx.enter_context(tc.tile_pool(name="psmm", bufs=2, space="PSUM"))
        psacc = ctx.enter_context(tc.tile_pool(name="psacc", bufs=2, space="PSUM"))

        # warm the ACT exp table set before anything depends on ACT
        warm = singles.tile([1, 4], F32, tag="warm", name="warm")
        nc.vector.memset(warm, 0.0)
        nc.scalar.activation(warm, warm, mybir.ActivationFunctionType.Exp)

        # ---- loads ----
        qk_sb = []  # [q0, k0, q1, k1], each [64, L]
        for i in range(4):
            t = singles.tile([64, L], BF16, tag=f"qk{i}", name=f"qk{i}")
            nc.sync.dma_start(out=t, in_=qk[i * 64 : (i + 1) * 64, :])
            qk_sb.append(t)
        va_sb = []
        for lt in range(NKT):
            t = singles.tile([128, 130], BF16, tag=f"va{lt}", name=f"va{lt}")
            nc.sync.dma_start(out=t, in_=va[lt * 128 : (lt + 1) * 128, :])
            va_sb.append(t)
        msk_sb = singles.tile([128, 128], BF16, tag="msk", name="msk_sb")
        nc.sync.dma_start(out=msk_sb, in_=msk)

        ot_sb = singles.tile([128, L], BF16, tag="ot", name="ot_sb")
        # separate 1-partition tiles: engine dests must start at a
        # partition offset that is a multiple of 32
        dd_sb = [
            singles.tile([1, L], F32, tag=f"dd{h}", name=f"dd{h}") for h in range(2)
        ]

        for h in range(2):
            q_t, k_t = qk_sb[2 * h], qk_sb[2 * h + 1]
            for hf in range(2):
                po = psacc.tile([65, HQ], F32, tag="acc", name="acc")
                for kt in range(NKT):
                    ps = psmm.tile([128, HQ], F32, tag="mm", name="mm")
                    for nt in range(2):
                        nc.tensor.matmul(
                            ps[:, nt * 512 : (nt + 1) * 512],
                            lhsT=k_t[:, kt * 128 : (kt + 1) * 128],
                            rhs=q_t[:, hf * HQ + nt * 512 : hf * HQ + (nt + 1) * 512],
                            start=True,
                            stop=True,
                        )
                    pt = ptp.tile([128, HQ], BF16, tag="pt", name="pt")
                    nc.scalar.activation(pt, ps, mybir.ActivationFunctionType.Exp)
                    if kt // 8 == hf:
                        off = (kt % 8) * 128
                        nc.vector.tensor_mul(
                            pt[:, off : off + 128], pt[:, off : off + 128], msk_sb
                        )
                    for nt in range(2):
                        nc.tensor.matmul(
                            po[:, nt * 512 : (nt + 1) * 512],
                            lhsT=va_sb[kt][:, h * 65 : (h + 1) * 65],
                            rhs=pt[:, nt * 512 : (nt + 1) * 512],
                            start=(kt == 0),
                            stop=(kt == NKT - 1),
                        )
                nc.scalar.copy(
                    ot_sb[h * 64 : (h + 1) * 64, hf * HQ : (hf + 1) * HQ], po[0:64, :]
                )
                nc.vector.tensor_copy(
                    dd_sb[h][:, hf * HQ : (hf + 1) * HQ], po[64:65, :]
                )
        nc.sync.dma_start(out=ot, in_=ot_sb)
        for h in range(2):
            nc.sync.dma_start(out=dd[h : h + 1, :], in_=dd_sb[h])


def _get_ctx():
    if _CTX:
        return _CTX
    nc = _build_nc()
    _b2j.install_neuronx_cc_hook()

    partition_name = nc.partition_id_tensor.name if nc.partition_id_tensor else None
    in_names, out_names, out_avals = [], [], []
    for alloc in nc.m.functions[0].allocations:
        if not isinstance(alloc, mybir.MemoryLocationSet):
            continue
        name = alloc.memorylocations[0].name
        if alloc.kind == "ExternalInput":
            if name != partition_name:
                in_names.append(name)
        elif alloc.kind == "ExternalOutput":
            out_names.append(name)
            out_avals.append(
                jax.core.ShapedArray(
                    tuple(alloc.tensor_shape), mybir.dt.np(alloc.dtype)
                )
            )
    n_params = len(in_names)
    in_names = in_names + out_names
    if partition_name is not None:
        in_names.append(partition_name)

    def _body(*args):
        operands = list(args)
        if partition_name is not None:
            operands.append(_b2j.partition_id_tensor())
        outs = _b2j._bass_exec_p.bind(
            *operands,
            out_avals=tuple(out_avals),
            in_names=tuple(in_names),
            out_names=tuple(out_names),
            lowering_input_output_aliases=(),
            sim_require_finite=True,
            sim_require_nnan=True,
            nc=nc,
        )
        return tuple(outs)

    devices = jax.devices()[:N_CORES]
    mesh = Mesh(np.asarray(devices), ("core",))
    n_ops = n_params + len(out_names)
    fn = jax.jit(
        shard_map(
            _body,
            mesh=mesh,
            in_specs=(P("core"),) * n_ops,
            out_specs=(P("core"),) * len(out_names),
            check_rep=False,
        ),
        keep_unused=True,
    )

    shd = NamedSharding(mesh, P("core"))
    # constants + output placeholder operands, device-resident across calls
    msk_g = np.tile((1.0 - np.eye(128, dtype=np.float32)).astype(BF), (N_CORES, 1))
    msk_d = jax.device_put(msk_g, shd)
    ot_ph = jax.device_put(np.zeros((N_CORES * 128, L), BF), shd)
    dd_ph = jax.device_put(np.zeros((N_CORES * 2, L), np.float32), shd)

    _CTX.update(
        nc=nc, fn=fn, shd=shd, msk_d=msk_d, ot_ph=ot_ph, dd_ph=dd_ph
    )
    return _CTX


def kernel(x, wq, wk, wv, wo):
    ctx = _get_ctx()
    x = np.asarray(x, dtype=np.float32)
    wq = np.asarray(wq, dtype=np.float32)
    wk = np.asarray(wk, dtype=np.float32)
    wv = np.asarray(wv, dtype=np.float32)
    wo = np.asarray(wo, dtype=np.float32)

    scale = 1.0 / (DH**0.5)
    x2 = x.reshape(B * L, D)  # (4096, 512)

    # qT/kT: [512 dims, 4096 tokens]; scale folded into q
    qT = (wq.T * scale) @ x2.T
    kT = wk.T @ x2.T

    # qk global: core c rows 256c..256c+256 = [q_h0^T, k_h0^T, q_h1^T, k_h1^T]
    qk_g = np.empty((N_CORES * 256, L), dtype=BF)
    for c in range(N_CORES):
        b, g = divmod(c, 4)
        cols = slice(b * L, (b + 1) * L)
        r = 256 * c
        qk_g[r : r + 64] = qT[128 * g : 128 * g + 64, cols]
        qk_g[r + 64 : r + 128] = kT[128 * g : 128 * g + 64, cols]
        qk_g[r + 128 : r + 192] = qT[128 * g + 64 : 128 * g + 128, cols]
        qk_g[r + 192 : r + 256] = kT[128 * g + 64 : 128 * g + 128, cols]
    qk_d = jax.device_put(qk_g, ctx["shd"])  # async; overlaps v GEMM below

    v = x2 @ wv  # (4096, 512)
    va_g = np.empty((N_CORES * L, 130), dtype=BF)
    for c in range(N_CORES):
        b, g = divmod(c, 4)
        rows = slice(c * L, (c + 1) * L)
        vb = v[b * L : (b + 1) * L]
        va_g[rows, 0:64] = vb[:, 128 * g : 128 * g + 64]
        va_g[rows, 65:129] = vb[:, 128 * g + 64 : 128 * g + 128]
    va_g[:, 64] = 1.0
    va_g[:, 129] = 1.0
    va_d = jax.device_put(va_g, ctx["shd"])

    ot_out, dd_out = ctx["fn"](qk_d, va_d, ctx["msk_d"], ctx["ot_ph"], ctx["dd_ph"])
    ot_g, dd_g = jax.device_get((ot_out, dd_out))
    ot_g = np.asarray(ot_g)
    dd_g = np.asarray(dd_g).astype(np.float32)

    # host epilogue: normalize by 1/d and project through wo
    y = np.empty((B, L, D), dtype=np.float32)
    rd = 1.0 / dd_g  # (16, 2048)
    for b in range(B):
        OT = np.empty((512, L), dtype=np.float32)  # dims x queries
        for g in range(4):
            c = 4 * b + g
            blk = ot_g[c * 128 : (c + 1) * 128].astype(np.float32)
            blk[0:64] *= rd[2 * c][None, :]
            blk[64:128] *= rd[2 * c + 1][None, :]
            OT[g * 128 : (g + 1) * 128] = blk
        y[b] = OT.T @ wo
    return y


# revision 5
# speedup vs baseline: 11.1002x; 2.3247x over previous
"""Diagonally-masked multi-head self-attention on 8 Trainium2 NeuronCores.

Problem (full shapes): x [2,2048,512], wq/wk/wv [512,512], wo [512,512],
H=8 heads, Dh=64.  out = softmax(mask_diag(q k^T / 8)) v @ wo.

The axon tunnel (~30-40MB/s each way, full duplex) dominates wall time,
so the design minimizes bytes moved (~5.3MB up, ~4.1MB down):

  upload: core c (batch b=c//4, head pair g=c%4) gets only
    - xq [512,512]  bf16: its quarter of x[b]^T (columns g*512..)
    - wh [256,384]  bf16: HALF of its head-pair weight pack
      [wq_h0*s | wk_h0 | wq_h1*s | wk_h1 | wv_h0 h1] (rows b*256..)
  device: AllGather xq over {4b..4b+3} -> full x[b]^T; AllGather wh
    over {c,c+4} -> full weight pack; then QKV projections, and
    attention per head:  S^T = K Q^T, exp on ACT (scores ~N(0,0.04),
    no max-subtraction needed), diagonal zeroed via (1-I) mask
    multiply, O'^T = V'^T P^T accumulated over 16 key tiles (V' has a
    ones column per head so row 64 is the softmax denominator d),
    normalized by 1/d via a DRAM-broadcast round trip.
  output: ot [128,2048] bf16 per core (O^T for its two heads) --
    disjoint across cores, no partial-sum all-reduce.
  host: only the final  y = O @ wo  GEMM (fp32 BLAS, ~20ms).

Dispatch bypasses run_bass_kernel_spmd: the jitted shard_map'd
bass_exec call is built once and cached; the (1-I) mask constant and
the output placeholder operand live on device permanently, so per-call
transfers are inputs+outputs only.
"""

import sys

if "/opt/trn_rl_repo" not in sys.path:
    sys.path.insert(0, "/opt/trn_rl_repo")

import numpy as np
import ml_dtypes

import jax
from jax.experimental.shard_map import shard_map
from jax.sharding import Mesh, NamedSharding, PartitionSpec as P

import concourse.bacc as bacc
import concourse.tile as tile
from concourse import mybir
from concourse import bass2jax as _b2j

N_CORES = 8
B, L, D = 2, 2048, 512
H, DH = 8, 64
HQ = L // 2  # 1024 queries per half
NKT = L // 128  # 16 key tiles
BF16 = mybir.dt.bfloat16
F32 = mybir.dt.float32
BF = ml_dtypes.bfloat16

# test.py compatibility
TRACE = False
_LAST_RESULTS = {}

_CTX = {}


def _build_nc():
    nc = bacc.Bacc(
        "TRN2",
        target_bir_lowering=False,
        debug=False,
        enable_asserts=False,
        num_devices=N_CORES,
    )
    xq = nc.dram_tensor("xq", [D, 512], BF16, kind="ExternalInput").ap()
    wh = nc.dram_tensor("wh", [256, 384], BF16, kind="ExternalInput").ap()
    msk = nc.dram_tensor("msk", [128, 128], BF16, kind="ExternalInput").ap()
    ot = nc.dram_tensor("ot", [128, L], BF16, kind="ExternalOutput").ap()
    with tile.TileContext(nc) as tc:
        _emit(nc, tc, xq, wh, msk, ot)
    nc.compile()
    return nc


def _emit(nc, tc, xq, wh, msk, ot):
    import contextlib

    ctx = contextlib.ExitStack()
    with ctx:
        singles = ctx.enter_context(tc.tile_pool(name="singles", bufs=1))
        ptp = ctx.enter_context(tc.tile_pool(name="pt", bufs=4))
        otmpp = ctx.enter_context(tc.tile_pool(name="otmpp", bufs=2))
        dbcp = ctx.enter_context(tc.tile_pool(name="dbcp", bufs=2))
        dram = ctx.enter_context(tc.tile_pool(name="dram", bufs=1, space="DRAM"))
        # PSUM budget (8 banks): psmm 2x[128,1024]=4 (S^T tiles + QK
        # projection), psacc 1x[65,1024]=2 (the O'^T accumulator),
        # psaux 2x[128,512]=2 (V projection).
        psmm = ctx.enter_context(tc.tile_pool(name="psmm", bufs=2, space="PSUM"))
        psacc = ctx.enter_context(tc.tile_pool(name="psacc", bufs=1, space="PSUM"))
        psaux = ctx.enter_context(tc.tile_pool(name="psaux", bufs=2, space="PSUM"))

        # warm the ACT exp table set before anything depends on ACT
        warm = singles.tile([1, 4], F32, tag="warm", name="warm")
        nc.vector.memset(warm, 0.0)
        nc.scalar.activation(warm, warm, mybir.ActivationFunctionType.Exp)

        # ---- gather x[b]^T and the full weight pack via NeuronLink ----
        xb = dram.tile([D, 512], BF16, tag="xb", name="xb")
        xg = dram.tile([4 * D, 512], BF16, tag="xg", name="xg")
        wb = dram.tile([256, 384], BF16, tag="wb", name="wb")
        wg = dram.tile([512, 384], BF16, tag="wg", name="wg")
        nc.gpsimd.dma_start(out=xb, in_=xq)
        nc.gpsimd.dma_start(out=wb, in_=wh)
        nc.gpsimd.collective_compute(
            "AllGather",
            mybir.AluOpType.bypass,
            replica_groups=[[0, 1, 2, 3], [4, 5, 6, 7]],
            ins=[xb.opt()],
            outs=[xg.opt()],
        )
        nc.gpsimd.collective_compute(
            "AllGather",
            mybir.AluOpType.bypass,
            replica_groups=[[0, 4], [1, 5], [2, 6], [3, 7]],
            ins=[wb.opt()],
            outs=[wg.opt()],
        )

        # ---- loads: xg rows j*512+kc*128 are xt[kc*128.., j*512..] ----
        wqk_sb = []
        wv_sb = []
        for kc in range(4):
            t = singles.tile([128, 256], BF16, tag=f"wqk{kc}", name=f"wqk{kc}")
            nc.sync.dma_start(out=t, in_=wg[kc * 128 : (kc + 1) * 128, 0:256])
            wqk_sb.append(t)
            t = singles.tile([128, 128], BF16, tag=f"wv{kc}", name=f"wv{kc}")
            nc.sync.dma_start(out=t, in_=wg[kc * 128 : (kc + 1) * 128, 256:384])
            wv_sb.append(t)
        xt_sb = [
            singles.tile([128, L], BF16, tag=f"xt{kc}", name=f"xt{kc}")
            for kc in range(4)
        ]
        for kc in range(4):
            for j in range(4):
                nc.sync.dma_start(
                    out=xt_sb[kc][:, j * 512 : (j + 1) * 512],
                    in_=xg[j * 512 + kc * 128 : j * 512 + (kc + 1) * 128, :],
                )
        msk_sb = singles.tile([128, 128], BF16, tag="msk", name="msk_sb")
        nc.sync.dma_start(out=msk_sb, in_=msk)

        # ---- QKV projections (fp32 PSUM accumulation over D) ----
        q_sb = [singles.tile([64, L], BF16, tag=f"q{h}", name=f"q{h}") for h in range(2)]
        k_sb = [singles.tile([64, L], BF16, tag=f"k{h}", name=f"k{h}") for h in range(2)]
        for h in range(2):
            for nt in range(4):
                ps = psmm.tile(
                    [128, 512], F32, tag="mm", name="qkps", padded_shape=[128, HQ]
                )
                for kc in range(4):
                    nc.tensor.matmul(
                        ps,
                        lhsT=wqk_sb[kc][:, h * 128 : (h + 1) * 128],
                        rhs=xt_sb[kc][:, nt * 512 : (nt + 1) * 512],
                        start=(kc == 0),
                        stop=(kc == 3),
                    )
                nc.vector.tensor_copy(q_sb[h][:, nt * 512 : (nt + 1) * 512], ps[0:64, :])
                nc.scalar.copy(k_sb[h][:, nt * 512 : (nt + 1) * 512], ps[64:128, :])

        va_sb = [
            singles.tile([128, 130], BF16, tag=f"va{lt}", name=f"va{lt}")
            for lt in range(NKT)
        ]
        for lt in range(NKT):
            ps = psaux.tile(
                [128, 128], F32, tag="aux", name="vps", padded_shape=[128, 512]
            )
            for kc in range(4):
                nc.tensor.matmul(
                    ps,
                    lhsT=xt_sb[kc][:, lt * 128 : (lt + 1) * 128],
                    rhs=wv_sb[kc],
                    start=(kc == 0),
                    stop=(kc == 3),
                )
            nc.vector.tensor_copy(va_sb[lt][:, 0:64], ps[:, 0:64])
            nc.vector.tensor_copy(va_sb[lt][:, 65:129], ps[:, 64:128])
            nc.vector.memset(va_sb[lt][:, 64:65], 1.0)
            nc.vector.memset(va_sb[lt][:, 129:130], 1.0)

        # ---- attention; O^T normalized by 1/d after a fast PSUM drain ----
        ot_all = singles.tile([128, L], BF16, tag="ot", name="ot_all")
        dscr = dram.tile([4, HQ], F32, tag="dscr", name="dscr")
        drow_sb = [
            singles.tile([1, HQ], F32, tag=f"dr{i}", name=f"dr{i}") for i in range(4)
        ]
        for h in range(2):
            for hf in range(2):
                po = psacc.tile([65, HQ], F32, tag="acc", name="acc")
                for kt in range(NKT):
                    ps = psmm.tile([128, HQ], F32, tag="mm", name="mm")
                    for nt in range(2):
                        nc.tensor.matmul(
                            ps[:, nt * 512 : (nt + 1) * 512],
                            lhsT=k_sb[h][:, kt * 128 : (kt + 1) * 128],
                            rhs=q_sb[h][
                                :, hf * HQ + nt * 512 : hf * HQ + (nt + 1) * 512
                            ],
                            start=True,
                            stop=True,
                        )
                    pt = ptp.tile([128, HQ], BF16, tag="pt", name="pt")
                    nc.scalar.activation(pt, ps, mybir.ActivationFunctionType.Exp)
                    if kt // 8 == hf:
                        off = (kt % 8) * 128
                        nc.vector.tensor_mul(
                            pt[:, off : off + 128], pt[:, off : off + 128], msk_sb
                        )
                    for nt in range(2):
                        nc.tensor.matmul(
                            po[:, nt * 512 : (nt + 1) * 512],
                            lhsT=va_sb[kt][:, h * 65 : (h + 1) * 65],
                            rhs=pt[:, nt * 512 : (nt + 1) * 512],
                            start=(kt == 0),
                            stop=(kt == NKT - 1),
                        )
                # fast drain so the accumulator frees quickly
                i = 2 * h + hf
                otmp = otmpp.tile([64, HQ], F32, tag="otmp", name="otmp")
                nc.scalar.copy(otmp, po[0:64, :])
                nc.vector.reciprocal(drow_sb[i], po[64:65, :])
                nc.sync.dma_start(out=dscr[i : i + 1, :], in_=drow_sb[i])
                rbc = dbcp.tile([64, HQ], F32, tag="rbc", name="rbc")
                nc.sync.dma_start(
                    out=rbc, in_=dscr[i : i + 1, :].to_broadcast([64, HQ])
                )
                nc.vector.tensor_mul(
                    ot_all[h * 64 : (h + 1) * 64, hf * HQ : (hf + 1) * HQ],
                    otmp,
                    rbc,
                )
        nc.sync.dma_start(out=ot, in_=ot_all)


def _get_ctx():
    if _CTX:
        return _CTX
    nc = _build_nc()
    _b2j.install_neuronx_cc_hook()

    partition_name = nc.partition_id_tensor.name if nc.partition_id_tensor else None
    in_names, out_names, out_avals = [], [], []
    for alloc in nc.m.functions[0].allocations:
        if not isinstance(alloc, mybir.MemoryLocationSet):
            continue
        name = alloc.memorylocations[0].name
        if alloc.kind == "ExternalInput":
            if name != partition_name:
                in_names.append(name)
        elif alloc.kind == "ExternalOutput":
            out_names.append(name)
            out_avals.append(
                jax.core.ShapedArray(
                    tuple(alloc.tensor_shape), mybir.dt.np(alloc.dtype)
                )
            )
    n_params = len(in_names)
    in_names = in_names + out_names
    if partition_name is not None:
        in_names.append(partition_name)

    def _body(*args):
        operands = list(args)
        if partition_name is not None:
            operands.append(_b2j.partition_id_tensor())
        outs = _b2j._bass_exec_p.bind(
            *operands,
            out_avals=tuple(out_avals),
            in_names=tuple(in_names),
            out_names=tuple(out_names),
            lowering_input_output_aliases=(),
            sim_require_finite=True,
            sim_require_nnan=True,
            nc=nc,
        )
        return tuple(outs)

    devices = jax.devices()[:N_CORES]
    mesh = Mesh(np.asarray(devices), ("core",))
    n_ops = n_params + len(out_names)
    fn = jax.jit(
        shard_map(
            _body,
            mesh=mesh,
            in_specs=(P("core"),) * n_ops,
            out_specs=(P("core"),) * len(out_names),
            check_rep=False,
        ),
        keep_unused=True,
    )

    shd = NamedSharding(mesh, P("core"))
    # constants + output placeholder operand, device-resident across calls
    msk_g = np.tile((1.0 - np.eye(128, dtype=np.float32)).astype(BF), (N_CORES, 1))
    msk_d = jax.device_put(msk_g, shd)
    ot_ph = jax.device_put(np.zeros((N_CORES * 128, L), BF), shd)

    _CTX.update(nc=nc, fn=fn, shd=shd, msk_d=msk_d, ot_ph=ot_ph)
    return _CTX


def kernel(x, wq, wk, wv, wo):
    ctx = _get_ctx()
    x = np.asarray(x, dtype=np.float32)
    wq = np.asarray(wq, dtype=np.float32)
    wk = np.asarray(wk, dtype=np.float32)
    wv = np.asarray(wv, dtype=np.float32)
    wo = np.asarray(wo, dtype=np.float32)

    scale = 1.0 / (DH**0.5)

    # xq global: core c rows c*512.. = x[b, g*512:(g+1)*512, :]^T
    xq_g = np.empty((N_CORES * D, 512), dtype=BF)
    for c in range(N_CORES):
        b, g = divmod(c, 4)
        xq_g[c * D : (c + 1) * D] = x[b, g * 512 : (g + 1) * 512, :].T
    xq_d = jax.device_put(xq_g, ctx["shd"])  # async; overlaps the packing below

    # wh global: per head pair g the pack [wq_h0*s|wk_h0|wq_h1*s|wk_h1|wv],
    # split in D-halves between cores g (rows 0:256) and g+4 (rows 256:512)
    wh_g = np.empty((N_CORES * 256, 384), dtype=BF)
    for g in range(4):
        h0 = 2 * g
        pack = np.concatenate(
            [
                wq[:, h0 * DH : (h0 + 1) * DH] * scale,
                wk[:, h0 * DH : (h0 + 1) * DH],
                wq[:, (h0 + 1) * DH : (h0 + 2) * DH] * scale,
                wk[:, (h0 + 1) * DH : (h0 + 2) * DH],
                wv[:, h0 * DH : (h0 + 2) * DH],
            ],
            axis=1,
        ).astype(BF)
        wh_g[g * 256 : (g + 1) * 256] = pack[0:256]
        wh_g[(g + 4) * 256 : (g + 5) * 256] = pack[256:512]
    wh_d = jax.device_put(wh_g, ctx["shd"])

    (ot_out,) = ctx["fn"](xq_d, wh_d, ctx["msk_d"], ctx["ot_ph"])
    ot_g = np.asarray(ot_out)

    # host epilogue: y = O @ wo
    y = np.empty((B, L, D), dtype=np.float32)
    for b in range(B):
        OT = np.empty((512, L), dtype=np.float32)  # dims x queries
        for g in range(4):
            c = 4 * b + g
            OT[g * 128 : (g + 1) * 128] = ot_g[c * 128 : (c + 1) * 128]
        y[b] = OT.T @ wo
    return y
